# revision 24
# baseline (speedup 1.0000x reference)
"""Trainium2 Bass kernel v4 (token-partition hybrid) for nn_BartPooler.

Layout: groups of G=8 consecutive tokens per partition row, [128, G*H] f16
tiles.  Per tile: VectorE computes a pair-sum (L1, fp16 2x mode) and a
3-level in-partition max tree; TensorE contracts the pair-sums against a
membership matrix (alpha and dup-token compensation folded into the
weights) for the segment means, and transposes the per-partition group
maxes; VectorE finishes the per-slot max over group columns.  Final GEMM:
16 k-blocks packed 4-up into PE column quadrants + fold, then bias+tanh.
"""

import numpy as np

import concourse.bacc as bacc
import concourse.mybir as mybir
import concourse.tile as tile
from concourse.bass_utils import run_bass_kernel_spmd
from concourse.masks import make_identity

NCORES = 8
B, S, H, T = 16, 4096, 1024, 16
D_OUT = 1024
HB = H // 128
G = 8            # tokens per partition row

F32 = mybir.dt.float32
F16 = mybir.dt.float16


def _groups_needed(cnt):
    g = -(-cnt // G)
    if cnt % G:
        g += 1       # ensure at least one pure-dup group for compensation
    return g


def _build_schedule(parts, turns):
    Bn, Tn = parts.shape
    segs = []
    for b in range(Bn):
        cum = 0
        for j in range(Tn):
            c = int(parts[b, j])
            if j < int(turns[b]):
                segs.append((b * Tn + j, b, 1 + cum, c))
            cum += c

    order = sorted(range(len(segs)), key=lambda i: -segs[i][3])
    core_slots = [[] for _ in range(NCORES)]
    for rank, i in enumerate(order):
        core_slots[rank % NCORES].append(segs[i])
    ncap = max(len(s) for s in core_slots)
    assert ncap <= 32, ncap

    NG = [max(_groups_needed(core_slots[c][j][3]) for c in range(NCORES)
              if j < len(core_slots[c])) for j in range(ncap)]
    AG = np.concatenate([[0], np.cumsum(NG)]).astype(np.int64)
    ngroups = int(AG[-1])
    ntiles = -(-ngroups // 128)
    ngpad = ntiles * 128
    ntok = ngpad * G

    tok_idx = np.zeros((NCORES, ntok), dtype=np.int64)
    mem = np.zeros((NCORES, 128, ntiles, ncap), dtype=np.float32)
    out_map = np.full((NCORES, ncap), -1, dtype=np.int64)
    for c in range(NCORES):
        for j, (grow, b, s0, cnt) in enumerate(core_slots[c]):
            out_map[c, j] = grow
            g0 = int(AG[j])
            lb = int(NG[j]) * G
            base = b * S + s0
            a = g0 * G
            tok_idx[c, a:a + cnt] = np.arange(base, base + cnt)
            tok_idx[c, a + cnt:a + lb] = base              # dup first token
            # membership weights: real/mixed groups 1/cnt; pure-dup groups
            # -r/(npure*G*cnt) so the r dups in the mixed group cancel.
            inv = 1.0 / cnt
            nfull, rem = divmod(cnt, G)
            nreal = nfull + (1 if rem else 0)
            npure = int(NG[j]) - nreal
            r = (G - rem) % G
            bw = -r / (npure * G) * inv if (npure and r) else 0.0
            for g in range(g0, g0 + nreal):
                mem[c, g % 128, g // 128, j] = inv
            for g in range(g0 + nreal, g0 + int(NG[j])):
                mem[c, g % 128, g // 128, j] = bw
    return {
        "ncap": ncap, "NG": NG, "AG": AG, "ngroups": ngroups,
        "ntiles": ntiles, "ntok": ntok,
        "tok_idx": tok_idx, "mem": mem,
        "out_map": out_map, "nrows": Bn * Tn,
    }


def _build_program(sched):
    ncap, ntiles = sched["ncap"], sched["ntiles"]
    AG, NG = sched["AG"], sched["NG"]

    nc = bacc.Bacc("TRN2", target_bir_lowering=False, debug=False,
                   num_devices=NCORES)
    hid = nc.dram_tensor("hid", [128, ntiles, G * H], F16,
                         kind="ExternalInput")
    mem = nc.dram_tensor("mem", [128, ntiles, ncap], F16,
                         kind="ExternalInput")
    wt = nc.dram_tensor("wt", [128, 4, 4, D_OUT], F16, kind="ExternalInput")
    brep = nc.dram_tensor("brep", [ncap, D_OUT], F32, kind="ExternalInput")
    fold = nc.dram_tensor("fold", [128, ncap], F16, kind="ExternalInput")
    out = nc.dram_tensor("out", [ncap, D_OUT], F32, kind="ExternalOutput")

    with tile.TileContext(nc) as tc:
        with (
            tc.tile_pool(name="main", bufs=1) as mp,
            tc.tile_pool(name="psum", bufs=1, space="PSUM") as pp,
            tc.tile_pool(name="trp", bufs=1, space="PSUM") as trpool,
        ):
            ident = mp.tile([128, 128], F16)
            make_identity(nc, ident[:])
            mem_sb = mp.tile([128, ntiles, ncap], F16)
            nc.scalar.dma_start(out=mem_sb[:], in_=mem[:])
            brep_sb = mp.tile([ncap, D_OUT], F32)
            nc.scalar.dma_start(out=brep_sb[:], in_=brep[:])
            fold_sb = mp.tile([128, ncap], F16)
            nc.scalar.dma_start(out=fold_sb[:], in_=fold[:])

            # Process the partial (smallest) tile FIRST: per-tile DVE cost is
            # free-size based, so this is free compute-wise but its smaller
            # DMA gets the pipeline started ~3us earlier.
            ngroups = sched["ngroups"]
            proc_order = ([ntiles - 1] + list(range(ntiles - 1))
                          if ntiles > 1 else [0])
            tiles = []
            for t in proc_order:
                pt = min(128, ngroups - t * 128)
                ht = mp.tile([128, G * H], F16, name=f"ht{t}", tag=f"ht{t}")
                nc.sync.dma_start(out=ht[:pt, :], in_=hid[:pt, t, :])
                tiles.append((t, ht, pt))
            wt_sb = mp.tile([128, 4, 4, D_OUT], F16)
            for step in range(4):
                nc.sync.dma_start(out=wt_sb[:, step], in_=wt[:, step])

            trmax = mp.tile([128, HB, ntiles * 128], F16)
            maxT = mp.tile([128, HB, ncap], F16)
            meansT = mp.tile([128, HB, ncap], F16)
            sum_ps = pp.tile([ncap, D_OUT], F32, name="sum_ps")

            # a slot's max reduce is ready once every tile its groups touch
            # has been processed (transposed into trmax)
            pos_of = {t: p for p, t in enumerate(proc_order)}
            cover = [[] for _ in range(ntiles)]
            for j in range(ncap):
                ta = int(AG[j]) // 128
                tb = (int(AG[j]) + int(NG[j]) - 1) // 128
                cover[max(pos_of[t] for t in range(ta, tb + 1))].append(j)

            def emit_cover_reduces(p):
                for j in cover[p]:
                    a, l = int(AG[j]), int(NG[j])
                    nc.vector.reduce_max(out=maxT[:, :, j:j + 1],
                                         in_=trmax[:, :, a:a + l],
                                         axis=mybir.AxisListType.X)

            for p, (t, ht, pt) in enumerate(tiles):
                half = G // 2 * H
                ssc = mp.tile([128, half], F16, name=f"ssc{t}", tag=f"ssc{t}")
                # sum L1 (token i + token i+G/2), then PE membership matmuls
                nc.vector.tensor_tensor(out=ssc[:pt, :], in0=ht[:pt, :half],
                                        in1=ht[:pt, half:],
                                        op=mybir.AluOpType.add)
                for pos in range(G // 2):
                    for nhh in range(2):
                        nc.tensor.matmul(
                            sum_ps[:, nhh * 512:(nhh + 1) * 512],
                            lhsT=mem_sb[:pt, t, :],
                            rhs=ssc[:pt, pos * H + nhh * 512:
                                    pos * H + nhh * 512 + 512],
                            start=(p == 0 and pos == 0),
                            stop=(p == ntiles - 1 and pos == G // 2 - 1),
                        )
                # max tree in place on ht: G -> 1 per partition
                m = G * H
                for _ in range(3):
                    nc.vector.tensor_tensor(out=ht[:pt, :m // 2],
                                            in0=ht[:pt, :m // 2],
                                            in1=ht[:pt, m // 2:m],
                                            op=mybir.AluOpType.max)
                    m //= 2
                # per-slot maxes for the PREVIOUS tile: by now its transpose
                # + copy have finished, so the in-order DVE queue does not
                # stall on the PE/Act chain of the current tile.
                if p > 0:
                    emit_cover_reduces(p - 1)
                # transpose gmax [pt, H] -> trmax group columns
                trp = trpool.tile([128, H], F16, tag="trp")
                for hb in range(HB):
                    nc.tensor.transpose(trp[:, hb * 128:hb * 128 + pt],
                                        ht[:pt, hb * 128:(hb + 1) * 128],
                                        ident[:pt, :pt])
                nc.scalar.copy(
                    out=trmax[:, :, t * 128:t * 128 + pt],
                    in_=trp[:].rearrange("p (b g) -> p b g", g=128)[:, :, :pt])
            emit_cover_reduces(ntiles - 1)

            # means: PSUM -> SBUF f16, transpose to [h, slot]
            means_s = mp.tile([ncap, D_OUT], F16)
            nc.scalar.copy(out=means_s[:], in_=sum_ps[:])
            tr2 = trpool.tile([128, HB * ncap], F16, tag="tr2")
            for hb in range(HB):
                nc.tensor.transpose(tr2[:, hb * ncap:(hb + 1) * ncap],
                                    means_s[:, hb * 128:(hb + 1) * 128],
                                    ident[:ncap, :ncap])
            nc.scalar.copy(out=meansT[:],
                           in_=tr2[:].rearrange("p (b j) -> p b j", j=ncap))

            # GEMM (4-up quadrant packing + fold), W chunk-pipelined
            osb = mp.tile([ncap, D_OUT], F32)
            for nh in range(2):
                nsl = slice(nh * 512, (nh + 1) * 512)
                gem_ps = pp.tile([128, 512], F32, name=f"gem{nh}")
                for step in range(4):
                    for cg in range(4):
                        kb = 4 * cg + step
                        lhsT = (maxT[:, kb, :] if kb < HB
                                else meansT[:, kb - HB, :])
                        nc.tensor.matmul(
                            gem_ps[32 * cg:32 * cg + ncap, :],
                            lhsT=lhsT,
                            rhs=wt_sb[:, step, cg, nsl],
                            start=(step == 0),
                            stop=(step == 3),
                            tile_position=(0, 32 * cg),
                        )
                gem_sb = mp.tile([128, 512], F16, name=f"gsb{nh}")
                nc.scalar.copy(out=gem_sb[:], in_=gem_ps[:])
                fold_ps = pp.tile([ncap, 512], F32, name=f"fps{nh}")
                nc.tensor.matmul(fold_ps[:], lhsT=fold_sb[:, :ncap],
                                 rhs=gem_sb[:], start=True, stop=True)
                # epilogue per 512-col half so half 0 streams out while
                # half 1 is still in the PE
                nc.vector.tensor_add(out=osb[:, nsl], in0=fold_ps[:],
                                     in1=brep_sb[:, nsl])
                nc.scalar.activation(osb[:, nsl], osb[:, nsl],
                                     mybir.ActivationFunctionType.Tanh)
                nc.sync.dma_start(out=out[:, nsl], in_=osb[:, nsl])

    nc.compile()
    return nc


def _build_in_maps(sched, hidden_states, W, b):
    ncap, ntiles, ntok = sched["ncap"], sched["ntiles"], sched["ntok"]
    flat16 = np.ascontiguousarray(
        np.asarray(hidden_states).reshape(B * S, H)).astype(np.float16)
    WT = np.asarray(W, dtype=np.float32).T.reshape(2 * HB, 128, D_OUT)
    wt_np = np.zeros((128, 4, 4, D_OUT), np.float32)
    for step in range(4):
        for cg in range(4):
            wt_np[:, step, cg, :] = WT[4 * cg + step]
    wt_np = np.ascontiguousarray(wt_np).astype(np.float16)
    brep_np = np.ascontiguousarray(
        np.broadcast_to(np.asarray(b, dtype=np.float32), (ncap, D_OUT)))
    fold_np = np.zeros((128, ncap), np.float16)
    for cg in range(4):
        for j in range(ncap):
            fold_np[32 * cg + j, j] = 1.0

    in_maps = []
    for c in range(NCORES):
        tok = flat16[sched["tok_idx"][c]]                # [ntok, H]
        hid_np = np.ascontiguousarray(
            tok.reshape(ntiles, 128, G * H).transpose(1, 0, 2))
        in_maps.append({
            "hid": hid_np,
            "mem": np.ascontiguousarray(sched["mem"][c]).astype(np.float16),
            "wt": wt_np,
            "brep": brep_np,
            "fold": fold_np,
        })
    return in_maps


def kernel(hidden_states, W, b, turns, parts):
    parts = np.asarray(parts)
    turns = np.asarray(turns)

    sched = _build_schedule(parts, turns)
    nc = _build_program(sched)
    in_maps = _build_in_maps(sched, hidden_states, W, b)

    res = run_bass_kernel_spmd(nc, in_maps, list(range(NCORES)))

    full = np.zeros((sched["nrows"], D_OUT), dtype=np.float32)
    for c in range(NCORES):
        oc = res.results[c]["out"]
        for j in range(sched["ncap"]):
            g = sched["out_map"][c, j]
            if g >= 0:
                full[g] = oc[j]
    return full
